# revision 2
# baseline (speedup 1.0000x reference)
"""Trainium2 Bass kernel for a 1-layer LSTM (B=2048, T=512, I=4, H=64) + FC (O=4).

Sharding: data-parallel over batch across 8 NeuronCores (256 examples/core);
the tiny LSTM/FC weights are replicated.

On-core layout ("transposed state"): SBUF partitions carry gate/hidden rows,
the free dimension carries batch.  The 256 local examples form two groups of
128; the two groups are stacked in the partition dimension (group 0 -> rows
0-63, group 1 -> rows 64-127) so ScalarE/VectorE instructions run with all
128 lanes busy and one instruction advances both groups.

Recurrent step t (lockstep over both groups, batch N=128 per group):
  z_g = [h_g (rows 0-63); ones (row 64); x_t^T (rows 65-68)]   # SBUF [69,128]
  8 matmuls (4 gate chunks x 2 groups), K=69, M=64, N=128:
      psA[128, 384] = [i | f | o]   (both groups stacked in partitions)
      psB[128, 128] = g-chunk
  sact = sigmoid(psA); tg = tanh(psB)          # 2 ScalarE instrs
  u = si*tg ; w = sf*c ; c = u + w             # 3 VectorE instrs [128,128]
  tc = tanh(c)                                 # 1 ScalarE instr
  h_g = so_g * tc_g  -> rows 0-63 of the other z buffer (group 1 needs a
      cross-quadrant partition shift, done as two 32-partition VectorE ops)

The input x is pre-transposed on the host to xT[T, I, B_local] so the
per-step x DMA is 4 contiguous rows.  Bias enters through the ones row of z;
the FC bias through the same ones row at the end.
"""

from contextlib import ExitStack

import numpy as np

import concourse.bass as bass
import concourse.tile as tile
from concourse import bacc, mybir
from concourse.bass_utils import run_bass_kernel_spmd

F32 = mybir.dt.float32
BF16 = mybir.dt.bfloat16
AF = mybir.ActivationFunctionType

H, I, O = 64, 4, 4
B, T_FULL = 2048, 512
NCORES = 8
BLOC = B // NCORES          # 256 examples per core
NG = 128                    # batch per group (2 groups per core)
KZ = H + 1 + I              # 69 rows of z: h, ones, x

# bf16 compute (matmuls, activations, cell state) keeps max rel err ~4e-3
# (measured against an f64 oracle) while roughly halving VectorE time.
USE_BF16 = True


def build_nc(T=T_FULL, use_bf16=None):
    if use_bf16 is None:
        use_bf16 = USE_BF16
    DT = BF16 if use_bf16 else F32
    nc = bacc.Bacc(
        "TRN2",
        target_bir_lowering=False,
        debug=False,
        enable_asserts=False,
        num_devices=NCORES,
    )

    xT = nc.dram_tensor("xT", [T, I, BLOC], DT, kind="ExternalInput")
    wz = nc.dram_tensor("wz", [KZ, 4, H], DT, kind="ExternalInput")
    wz2 = nc.dram_tensor("wz2", [2 * H, 4, H], DT, kind="ExternalInput")
    wfc = nc.dram_tensor("wfc", [KZ, O], DT, kind="ExternalInput")
    wfc2 = nc.dram_tensor("wfc2", [2 * H, O], DT, kind="ExternalInput")
    out = nc.dram_tensor("out", [2, O, NG], F32, kind="ExternalOutput")

    with tile.TileContext(nc) as tc, ExitStack() as ctx:
        persist = ctx.enter_context(tc.tile_pool(name="persist", bufs=1))
        acts = ctx.enter_context(tc.tile_pool(name="acts", bufs=3))
        temps = ctx.enter_context(tc.tile_pool(name="temps", bufs=3))
        psum = ctx.enter_context(tc.tile_pool(name="psum", bufs=2, space="PSUM"))

        wz_sb = persist.tile([KZ, 4, H], DT, tag="wz")
        nc.sync.dma_start(wz_sb[:], wz[:])
        wz2_sb = persist.tile([2 * H, 4, H], DT, tag="wz2")
        nc.sync.dma_start(wz2_sb[:], wz2[:])
        wfc_sb = persist.tile([KZ, O], DT, tag="wfc")
        nc.sync.dma_start(wfc_sb[:], wfc[:])
        wfc2_sb = persist.tile([2 * H, O], DT, tag="wfc2")
        nc.sync.dma_start(wfc2_sb[:], wfc2[:])

        # Persistent state: cell state (both groups stacked) and the two
        # double-buffered z tiles per group.  Group 0's z is [h; 1; x] (K=69,
        # h in partitions 0-63); group 1's is [1; x; zeros; h] (K=128, h in
        # partitions 64-127, zero rows cost nothing on the PE) so BOTH h
        # updates write the same partitions their operands live in.
        c_st = persist.tile([2 * H, NG], DT, tag="c")
        nc.vector.memset(c_st[:], 0.0)
        zbuf = []
        for j in range(2):
            z = persist.tile([KZ, NG], DT, tag=f"z0{j}")
            nc.vector.memset(z[0:H, :], 0.0)        # h0 = 0
            nc.vector.memset(z[H : H + 1, :], 1.0)  # ones row
            zbuf.append(z)
        zbuf2 = []
        for j in range(2):
            z = persist.tile([2 * H, NG], DT, tag=f"z1{j}")
            nc.vector.memset(z[:], 0.0)             # zeros rows + h0 = 0
            nc.vector.memset(z[0:1, :], 1.0)        # ones row (row 0)
            zbuf2.append(z)

        # Schedule rationale (latency-bound loop; see trace analysis):
        #   - group 1's h-mul issues FIRST on DVE, so its z is ready first;
        #     the MM queue leads with group-1 chunks, interleaving group 0's
        #     behind them (group 0's h-mul completes during the early MMs).
        #   - g-chunks go LAST on PE: sigmoid(ifo) gates the c-update and
        #     should start as early as possible; tanh(g) follows sigmoid on
        #     ScalarE and its result arrives just in time for u = si*tg.
        #   - DVE order w (needs sigmoid only), u (needs tanh g), add.
        for t in range(T):
            zc = [zbuf[t % 2], zbuf2[t % 2]]
            zn = [zbuf[(t + 1) % 2], zbuf2[(t + 1) % 2]]

            # x_t for this step (prefetched ~1 step ahead by the sync queue)
            nc.sync.dma_start(zc[1][1 : 1 + I, :], xT[t, :, NG : 2 * NG])
            nc.sync.dma_start(zc[0][H + 1 : KZ, :], xT[t, :, 0:NG])

            psA = psum.tile([2 * H, 3 * NG], F32, tag="psA")  # [i | f | o]
            psB = psum.tile([2 * H, NG], F32, tag="psB")      # g-chunk
            wzs = [wz_sb, wz2_sb]
            for ci, ch in enumerate((0, 1, 3)):  # i, f, o chunks, group 1
                nc.tensor.matmul(
                    psA[H : 2 * H, ci * NG : (ci + 1) * NG],
                    wz2_sb[:, ch, :],
                    zc[1][:],
                    start=True,
                    stop=True,
                )
            for ci, ch in enumerate((0, 1, 3)):  # i, f, o chunks, group 0
                nc.tensor.matmul(
                    psA[0:H, ci * NG : (ci + 1) * NG],
                    wz_sb[:, ch, :],
                    zc[0][:],
                    start=True,
                    stop=True,
                )
            nc.tensor.matmul(
                psB[H : 2 * H, :], wz2_sb[:, 2, :], zc[1][:], start=True, stop=True
            )
            nc.tensor.matmul(
                psB[0:H, :], wz_sb[:, 2, :], zc[0][:], start=True, stop=True
            )

            sact = acts.tile([2 * H, 3 * NG], DT, tag="sact")
            nc.scalar.activation(sact[:], psA[:], AF.Sigmoid)
            tg = acts.tile([2 * H, NG], DT, tag="tg")
            nc.scalar.activation(tg[:], psB[:], AF.Tanh)

            si = sact[:, 0:NG]
            sf = sact[:, NG : 2 * NG]
            so = sact[:, 2 * NG : 3 * NG]

            w = temps.tile([2 * H, NG], DT, tag="w")
            nc.vector.tensor_mul(w[:], sf, c_st[:])
            u = temps.tile([2 * H, NG], DT, tag="u")
            nc.vector.tensor_mul(u[:], si, tg[:])
            nc.vector.tensor_add(c_st[:], u[:], w[:])

            tcs = acts.tile([2 * H, NG], DT, tag="tc")
            nc.scalar.activation(tcs[:], c_st[:], AF.Tanh)

            # h updates: each group writes the partitions it already lives in;
            # group 1 first so the next step's leading MMs unblock sooner.
            nc.vector.tensor_mul(zn[1][H : 2 * H, :], so[H:, :], tcs[H:, :])
            nc.vector.tensor_mul(zn[0][0:H, :], so[0:H, :], tcs[0:H, :])

        # Final FC (bias added on the host during the gather).
        zf0, zf1 = zbuf[T % 2], zbuf2[T % 2]
        fc_ps0 = psum.tile([O, NG], F32, tag="fc0")
        nc.tensor.matmul(fc_ps0[:], wfc_sb[:], zf0[:], start=True, stop=True)
        fc_ps1 = psum.tile([O, NG], F32, tag="fc1")
        nc.tensor.matmul(fc_ps1[:], wfc2_sb[:], zf1[:], start=True, stop=True)
        for g, fc_ps in enumerate((fc_ps0, fc_ps1)):
            fc_sb = temps.tile([O, NG], F32, tag="fcsb")
            nc.vector.tensor_copy(fc_sb[:], fc_ps[:])
            nc.sync.dma_start(out[g], fc_sb[:])

    nc.compile()
    return nc


def prep_weights(W_ih, W_hh, b_ih, b_hh, W_fc, b_fc):
    bsum = (b_ih + b_hh).astype(np.float32)
    # group 0 z rows: [h (64); ones (1); x (4)] -> [W_hh^T; b; W_ih^T]
    wz = np.empty((KZ, 4, H), np.float32)
    # group 1 z rows: [ones (1); x (4); zeros (59); h (64)]
    wz2 = np.zeros((2 * H, 4, H), np.float32)
    for ch in range(4):
        r = slice(ch * H, (ch + 1) * H)
        wz[0:H, ch, :] = W_hh[r].T
        wz[H, ch, :] = bsum[r]
        wz[H + 1 :, ch, :] = W_ih[r].T
        wz2[0, ch, :] = bsum[r]
        wz2[1 : 1 + I, ch, :] = W_ih[r].T
        wz2[H:, ch, :] = W_hh[r].T
    wfc = np.zeros((KZ, O), np.float32)
    wfc[0:H] = W_fc.T
    wfc2 = np.zeros((2 * H, O), np.float32)
    wfc2[H:] = W_fc.T
    return wz, wz2, wfc, wfc2


def make_in_maps(x, W_ih, W_hh, b_ih, b_hh, W_fc, b_fc, T=T_FULL, use_bf16=None):
    import ml_dtypes

    if use_bf16 is None:
        use_bf16 = USE_BF16
    npdt = ml_dtypes.bfloat16 if use_bf16 else np.float32
    wz, wz2, wfc, wfc2 = prep_weights(W_ih, W_hh, b_ih, b_hh, W_fc, b_fc)
    wz, wz2, wfc, wfc2 = (a.astype(npdt) for a in (wz, wz2, wfc, wfc2))
    in_maps = []
    for core in range(NCORES):
        xc = x[core * BLOC : (core + 1) * BLOC, :T, :]  # [BLOC, T, I]
        xTc = np.ascontiguousarray(xc.transpose(1, 2, 0)).astype(npdt)
        in_maps.append({"xT": xTc, "wz": wz, "wz2": wz2, "wfc": wfc, "wfc2": wfc2})
    return in_maps


_CACHED_NC = None


def kernel(x, W_ih, W_hh, b_ih, b_hh, W_fc, b_fc):
    global _CACHED_NC
    x = np.asarray(x, np.float32)
    args = [np.asarray(a, np.float32) for a in (W_ih, W_hh, b_ih, b_hh, W_fc, b_fc)]
    if _CACHED_NC is None:
        _CACHED_NC = build_nc()
    nc = _CACHED_NC
    in_maps = make_in_maps(x, *args)
    res = run_bass_kernel_spmd(nc, in_maps, core_ids=list(range(NCORES)))
    b_fc = args[5]
    full = np.empty((1, B, O), np.float32)
    for core in range(NCORES):
        oc = res.results[core]["out"]  # [2, O, NG]
        for g in range(2):
            lo = core * BLOC + g * NG
            full[0, lo : lo + NG, :] = oc[g].T + b_fc
    return full

